# revision 1
# baseline (speedup 1.0000x reference)
"""MoE (top-1 routing, E=8) Trainium2 Bass kernel.

Full-input contract: kernel(**inputs) takes the unsharded numpy inputs of
reference.setup_inputs() and returns the full [N, H] float32 output.

Strategy (token-parallel SPMD over 8 NeuronCores, 2048 tokens/core):
  host:   fp32 router (x @ Wr.T + br, argmax); per-core expert-sorted
          "capacity layout" (per-slot 128-aligned segments, experts
          permuted per core by descending count so one static capacity
          profile fits every core); tokens pre-gathered into the sorted
          layout; fp16 casts; weights/activations pre-tiled so every
          device DMA is one large fully-contiguous transfer with the
          contraction dim D on SBUF partitions.
  device: shared FFN = dense fp16 matmuls over token-order tiles
          (tokens stationary, H moving in 512-slices, K=8x128 chunks
          accumulated in PSUM fp32) + bs -> fp16 "out" rows.
          routed FFN = fp16 matmuls over the sorted capacity tiles with
          each slot's expert weights + be -> fp16 "routed" rows (dense,
          sorted layout).
  host:   out[token] = shared[token] + routed[slot_of(token)], fp32.
"""

import sys

sys.path.insert(0, "/opt/trn_rl_repo")

from dataclasses import dataclass

import numpy as np

import concourse.bass as bass
import concourse.mybir as mybir
from concourse.tile import TileContext

# ----------------------------------------------------------------------------
# configuration
# ----------------------------------------------------------------------------


@dataclass
class Cfg:
    n_loc: int = 2048          # tokens per core
    d: int = 1024              # input dim (contraction)
    h: int = 4096              # hidden dim
    e: int = 8                 # experts
    cap: tuple = (3, 3, 3, 3, 3, 2, 2, 2)  # tiles per sorted slot
    n_cores: int = 8

    @property
    def kc(self):
        return self.d // 128

    @property
    def ns(self):
        return self.h // 512

    @property
    def nt(self):
        return self.n_loc // 128

    @property
    def rt(self):
        return sum(self.cap)


F16 = mybir.dt.float16
F32 = mybir.dt.float32

MAX_WAITS = 1


def split_long_waits(nc, max_w: int = MAX_WAITS):
    """walrus TPB_CTRL codegen rejects instructions with multiple sync waits
    (CoreV3GenImpl setupSyncWait).  Tile's exit drain can exceed that; move
    excess waits onto same-engine NoOps inserted just before the offender."""
    n_fix = 0
    for f in nc.m.functions:
        for bb in f.blocks:
            insts = bb.instructions
            new_list = []
            changed = False
            for inst in insts:
                si = inst.sync_info
                if si is not None and len(si.on_wait) > max_w:
                    w = list(si.on_wait)
                    k = 0
                    while len(w) > max_w:
                        chunk, w = w[:max_w], w[max_w:]
                        nop = mybir.InstNoOp(
                            name=f"{inst.name}_waitsplit_{k}",
                            engine=inst.engine,
                            sync_info=mybir.SyncInfo(on_wait=chunk, on_update=[]),
                            bass_nofuse=True,
                        )
                        new_list.append(nop)
                        k += 1
                    inst.sync_info = mybir.SyncInfo(
                        on_wait=w, on_update=list(si.on_update)
                    )
                    n_fix += 1
                    changed = True
                new_list.append(inst)
            if changed:
                bb.instructions = new_list
    return n_fix


# ----------------------------------------------------------------------------
# device program
# ----------------------------------------------------------------------------


def build_program(cfg: Cfg, fix_waits: bool = True):
    nc = bass.Bass()

    # all activation/weight params are pre-tiled on the host so that each
    # DMA below is a single fully-contiguous transfer.
    xt = nc.declare_dram_parameter(
        "xt16", [128, cfg.kc * cfg.n_loc], F16, isOutput=False
    )
    xg_d = nc.declare_dram_parameter(
        "xg16", [128, cfg.kc * cfg.rt * 128], F16, isOutput=False
    )
    wst = nc.declare_dram_parameter(
        "wst16", [128, cfg.kc * cfg.h], F16, isOutput=False
    )
    wet = nc.declare_dram_parameter(
        "wet16", [cfg.e * cfg.ns, 128, cfg.kc * 512], F16, isOutput=False
    )
    bsr = nc.declare_dram_parameter("bs_rep", [128, cfg.h], F16, isOutput=False)
    ber = nc.declare_dram_parameter(
        "be_rep", [cfg.e, 128, cfg.h], F16, isOutput=False
    )
    outp = nc.declare_dram_parameter("out", [cfg.n_loc, cfg.h], F16, isOutput=True)
    routp = nc.declare_dram_parameter(
        "routed", [cfg.rt * 128, cfg.h], F16, isOutput=True
    )

    base = np.cumsum([0] + list(cfg.cap))  # slot -> first tile index
    cap_max = max(cfg.cap)

    with TileContext(nc) as tc:
        with (
            tc.tile_pool(name="resident", bufs=1) as rpool,
            tc.tile_pool(name="wrt", bufs=3) as wepool,
            tc.tile_pool(name="ps", bufs=8, space="PSUM") as pspool,
        ):
            # ---- resident loads (each one contiguous DMA) -----------------
            xg = rpool.tile([128, cfg.kc, cfg.rt * 128], F16, tag="xg")

            # ---- shared FFN: dense token-order tiles, full-row stores -----
            with (
                tc.tile_pool(name="sh_res", bufs=1) as spool,
                tc.tile_pool(name="oshared", bufs=2) as opool,
            ):
                xts = spool.tile([128, cfg.kc, cfg.n_loc], F16, tag="xts")
                wsall = spool.tile([128, cfg.kc, cfg.h], F16, tag="ws")
                hn, hh = cfg.n_loc // 2, cfg.h // 2
                for k in range(cfg.kc):
                    for half in range(2):
                        nc.sync.dma_start(
                            out=xts[:, k, half * hn : (half + 1) * hn],
                            in_=xt[
                                :,
                                k * cfg.n_loc + half * hn : k * cfg.n_loc
                                + (half + 1) * hn,
                            ],
                        )
                        nc.sync.dma_start(
                            out=wsall[:, k, half * hh : (half + 1) * hh],
                            in_=wst[
                                :,
                                k * cfg.h + half * hh : k * cfg.h
                                + (half + 1) * hh,
                            ],
                        )
                bs_sb = spool.tile([128, cfg.h], F16, tag="bs")
                nc.sync.dma_start(out=bs_sb[:, :], in_=bsr[:, :])
                # sorted tokens for the routed pass: issued after the shared
                # residents so it overlaps shared compute instead of
                # delaying it (same HWDGE FIFO ring).
                nc.sync.dma_start(out=xg[:, :, :], in_=xg_d[:, :])

                for t in range(cfg.nt):
                    sh = opool.tile([128, cfg.h], F16, tag="osh")
                    # k-outer: one stationary (token tile, k-chunk) feeds all
                    # 8 H-slices -> 8x fewer LDWEIGHTS on the weight port.
                    pss = []
                    for _n in range(cfg.ns):
                        ps_n = pspool.tile(
                            [128, 512], F32, tag="ps", name=f"ps_{t}_{_n}"
                        )
                        pss.append(ps_n)
                    for k in range(cfg.kc):
                        for n in range(cfg.ns):
                            nc.tensor.matmul(
                                pss[n][:, :],
                                lhsT=xts[:, k, t * 128 : (t + 1) * 128],
                                rhs=wsall[:, k, n * 512 : (n + 1) * 512],
                                start=(k == 0),
                                stop=(k == cfg.kc - 1),
                            )
                    for n in range(cfg.ns):
                        nc.vector.tensor_add(
                            out=sh[:, n * 512 : (n + 1) * 512],
                            in0=pss[n][:, :],
                            in1=bs_sb[:, n * 512 : (n + 1) * 512],
                        )
                    nc.scalar.dma_start(
                        out=outp[t * 128 : (t + 1) * 128, :], in_=sh[:, :]
                    )

            # ---- routed FFN: sorted capacity layout -----------------------
            with (
                tc.tile_pool(name="stage", bufs=2) as stpool,
                tc.tile_pool(name="bias_e", bufs=2) as bpool,
            ):
                for s in range(cfg.e):
                    bes = bpool.tile([128, cfg.h], F16, tag="be")
                    nc.sync.dma_start(out=bes[:, :], in_=ber[s, :, :])
                    st = stpool.tile([128, cap_max, cfg.h], F16, tag="st")
                    for n in range(cfg.ns):
                        wtile = wepool.tile([128, cfg.kc, 512], F16, tag="we")
                        nc.sync.dma_start(
                            out=wtile[:, :, :], in_=wet[s * cfg.ns + n, :, :]
                        )
                        for tl in range(cfg.cap[s]):
                            t = base[s] + tl
                            ps = pspool.tile([128, 512], F32, tag="ps")
                            for k in range(cfg.kc):
                                nc.tensor.matmul(
                                    ps[:, :],
                                    lhsT=xg[:, k, t * 128 : (t + 1) * 128],
                                    rhs=wtile[:, k, :],
                                    start=(k == 0),
                                    stop=(k == cfg.kc - 1),
                                )
                            nc.vector.tensor_add(
                                out=st[:, tl, n * 512 : (n + 1) * 512],
                                in0=ps[:, :],
                                in1=bes[:, n * 512 : (n + 1) * 512],
                            )
                    for tl in range(cfg.cap[s]):
                        t = base[s] + tl
                        nc.scalar.dma_start(
                            out=routp[t * 128 : (t + 1) * 128, :],
                            in_=st[:, tl, :],
                        )

    if fix_waits:
        split_long_waits(nc)
    return nc


# ----------------------------------------------------------------------------
# host-side routing / input prep / combine
# ----------------------------------------------------------------------------


def _part_tile(a, kc):
    """[kc*128, F] -> [128, kc*F] with [p, k*F+j] = a[k*128+p, j]."""
    f = a.shape[1]
    return a.reshape(kc, 128, f).transpose(1, 0, 2).reshape(128, kc * f)


def route_and_pack(cfg: Cfg, te):
    """Per-core routing tables.  te [n_loc] expert ids.

    Returns (perm, sorted_tokens, valid): sorted_tokens [rt*128] maps
    capacity slot -> token id (pad slots -> token 0, valid False)."""
    counts = np.bincount(te, minlength=cfg.e)
    perm = np.argsort(-counts, kind="stable")
    base = np.cumsum([0] + list(cfg.cap))
    sorted_tokens = np.zeros(cfg.rt * 128, dtype=np.int64)
    valid = np.zeros(cfg.rt * 128, dtype=bool)
    for s in range(cfg.e):
        ex = perm[s]
        toks = np.nonzero(te == ex)[0]
        assert len(toks) <= cfg.cap[s] * 128, (
            f"slot {s} expert {ex}: {len(toks)} tokens > capacity "
            f"{cfg.cap[s] * 128}"
        )
        off = base[s] * 128
        sorted_tokens[off : off + len(toks)] = toks
        valid[off : off + len(toks)] = True
    return perm, sorted_tokens, valid


def make_in_map(cfg: Cfg, xs, te, Ws, bs, We, be):
    perm, sorted_tokens, valid = route_and_pack(cfg, te)
    x16 = np.ascontiguousarray(xs).astype(np.float16)
    xT = np.ascontiguousarray(xs.T).astype(np.float16)          # [d, n_loc]
    xgT = np.ascontiguousarray(x16[sorted_tokens].T)            # [d, rt*128]
    WsT = np.ascontiguousarray(Ws.T).astype(np.float16)         # [d, h]
    # routed weights pre-tiled per (slot, n): [e*ns, 128, kc*512] with
    # [s*ns+n, p, k*512+j] = We[perm[s]].T[k*128+p, n*512+j]
    WeT = We[perm].transpose(0, 2, 1).astype(np.float16)        # [e, d, h]
    wet = (
        WeT.reshape(cfg.e, cfg.kc, 128, cfg.ns, 512)
        .transpose(0, 3, 2, 1, 4)
        .reshape(cfg.e * cfg.ns, 128, cfg.kc * 512)
    )
    return {
        "xt16": _part_tile(xT, cfg.kc),
        "xg16": _part_tile(xgT, cfg.kc),
        "wst16": _part_tile(WsT, cfg.kc),
        "wet16": np.ascontiguousarray(wet),
        "bs_rep": np.ascontiguousarray(
            np.broadcast_to(bs.astype(np.float16), (128, cfg.h))
        ),
        "be_rep": np.ascontiguousarray(
            np.broadcast_to(
                be[perm].astype(np.float16)[:, None, :], (cfg.e, 128, cfg.h)
            )
        ),
    }, (sorted_tokens, valid)


def combine(cfg: Cfg, shared_out, routed_out, sorted_tokens, valid):
    out = shared_out.astype(np.float32)
    out[sorted_tokens[valid]] += routed_out[valid].astype(np.float32)
    return out


# ----------------------------------------------------------------------------
# entry point
# ----------------------------------------------------------------------------

_PROGRAM_CACHE = {}


def _get_program(cfg: Cfg):
    key = (cfg.n_loc, cfg.d, cfg.h, cfg.e, cfg.cap)
    if key not in _PROGRAM_CACHE:
        _PROGRAM_CACHE[key] = build_program(cfg)
    return _PROGRAM_CACHE[key]


def kernel(x, Ws, bs, We, be, Wr, br):
    from concourse.bass_utils import run_bass_kernel_spmd

    cfg = Cfg()
    x = np.asarray(x, dtype=np.float32)
    Ws = np.asarray(Ws, dtype=np.float32)
    bs = np.asarray(bs, dtype=np.float32)
    We = np.asarray(We, dtype=np.float32)
    be = np.asarray(be, dtype=np.float32)
    Wr = np.asarray(Wr, dtype=np.float32)
    br = np.asarray(br, dtype=np.float32)

    n = x.shape[0]
    assert n == cfg.n_loc * cfg.n_cores

    logits = x @ Wr.T + br
    te = np.argmax(logits, axis=-1)

    nc = _get_program(cfg)
    in_maps, metas = [], []
    for c in range(cfg.n_cores):
        sl = slice(c * cfg.n_loc, (c + 1) * cfg.n_loc)
        m, meta = make_in_map(cfg, x[sl], te[sl], Ws, bs, We, be)
        in_maps.append(m)
        metas.append(meta)

    res = run_bass_kernel_spmd(nc, in_maps, list(range(cfg.n_cores)))
    outs = []
    for c in range(cfg.n_cores):
        st, valid = metas[c]
        outs.append(
            combine(cfg, res.results[c]["out"], res.results[c]["routed"], st, valid)
        )
    return np.concatenate(outs, axis=0)



# revision 2
# speedup vs baseline: 2.2112x; 2.2112x over previous
"""MoE (top-1 routing, E=8) Trainium2 Bass kernel — merged-weight version.

Full-input contract: kernel(**inputs) takes the unsharded numpy inputs of
reference.setup_inputs() and returns the full [N, H] float32 output.

Algebraic core: top-1 routing PARTITIONS tokens by expert, so for a token
routed to expert e,

    out = x @ Ws.T + bs + x @ We[e].T + be[e]
        = x @ (Ws + We[e]).T + (bs + be[e])

i.e. the shared and routed FFNs merge into ONE matmul against per-expert
pre-summed weights (summed on host, free).  Device FLOPs halve vs the
two-pass formulation.

Sharding (8 cores): expert-parallel with load balancing.  The global
expert tile demands (ceil(count_e/128), sum ~131) are bin-packed into
8 cores x two weight slots (slot A: 9 tiles, slot B: 8 tiles) so every
core runs the same static program of T=17 token-tiles: tiles 0..8 use
slot A's resident merged weight, tiles 9..16 slot B's.  Which expert a
slot holds is pure input binding, so one compiled program serves any
routing that fits the (9,8) capacity profile; a fallback solver widens
T if needed (different cache key, still correct).

Device schedule per tile: n-major over 8 H-slices of 512, k-inner over
8 D-chunks of 128 accumulated in PSUM fp32 (measured 216 ns/matmul
sustained, LDWEIGHTS hidden), bias add on DVE, fp16 row stores.

Host: router (tiny), expert sort/pack, fp16 casts + pre-tiling, final
row scatter.  No arithmetic combine on host — device rows ARE the
output rows.
"""

import sys

sys.path.insert(0, "/opt/trn_rl_repo")

from dataclasses import dataclass

import numpy as np

import concourse.bass as bass
import concourse.mybir as mybir
from concourse.tile import TileContext

# ----------------------------------------------------------------------------
# configuration
# ----------------------------------------------------------------------------

N, D, H, E = 16384, 1024, 4096, 8
N_CORES = 8
KC = D // 128          # contraction chunks
NS = H // 512          # output H-slices


@dataclass(frozen=True)
class Cfg:
    a: int = 9             # tiles served by weight slot A
    b: int = 8             # tiles served by weight slot B

    @property
    def t(self):
        return self.a + self.b


F16 = mybir.dt.float16
F32 = mybir.dt.float32

MAX_WAITS = 1


def split_long_waits(nc, max_w: int = MAX_WAITS):
    """walrus TPB_CTRL codegen rejects instructions with multiple sync waits
    (CoreV3GenImpl setupSyncWait).  Tile's exit drain can exceed that; move
    excess waits onto same-engine NoOps inserted just before the offender."""
    n_fix = 0
    for f in nc.m.functions:
        for bb in f.blocks:
            insts = bb.instructions
            new_list = []
            changed = False
            for inst in insts:
                si = inst.sync_info
                if si is not None and len(si.on_wait) > max_w:
                    w = list(si.on_wait)
                    k = 0
                    while len(w) > max_w:
                        chunk, w = w[:max_w], w[max_w:]
                        nop = mybir.InstNoOp(
                            name=f"{inst.name}_waitsplit_{k}",
                            engine=inst.engine,
                            sync_info=mybir.SyncInfo(on_wait=chunk, on_update=[]),
                            bass_nofuse=True,
                        )
                        new_list.append(nop)
                        k += 1
                    inst.sync_info = mybir.SyncInfo(
                        on_wait=w, on_update=list(si.on_update)
                    )
                    n_fix += 1
                    changed = True
                new_list.append(inst)
            if changed:
                bb.instructions = new_list
    return n_fix


# ----------------------------------------------------------------------------
# device program
# ----------------------------------------------------------------------------


def build_program(cfg: Cfg, fix_waits: bool = True):
    nc = bass.Bass()

    # weights per slot, n-major so compute can start after one 1 MB chunk:
    # [ns, 128, kc*512] with [n, p, k*512+j] = (Ws+We[e]).T[k*128+p, n*512+j]
    wa_d = nc.declare_dram_parameter("wa16", [NS, 128, KC * 512], F16, isOutput=False)
    wb_d = nc.declare_dram_parameter("wb16", [NS, 128, KC * 512], F16, isOutput=False)
    ba_d = nc.declare_dram_parameter("ba16", [128, H], F16, isOutput=False)
    bb_d = nc.declare_dram_parameter("bb16", [128, H], F16, isOutput=False)
    # token tiles: [t, p, k*128+m] = x[tok[t*128+m], k*128+p]
    xga_d = nc.declare_dram_parameter("xga16", [cfg.a, 128, KC * 128], F16, isOutput=False)
    xgb_d = nc.declare_dram_parameter("xgb16", [cfg.b, 128, KC * 128], F16, isOutput=False)
    out_d = nc.declare_dram_parameter("out", [cfg.t * 128, H], F16, isOutput=True)

    with TileContext(nc) as tc:
        with (
            tc.tile_pool(name="wres", bufs=1) as wpool,
            tc.tile_pool(name="xstream", bufs=3) as xpool,
            tc.tile_pool(name="ostage", bufs=2) as opool,
            tc.tile_pool(name="ps", bufs=4, space="PSUM") as pspool,
        ):
            wa = wpool.tile([128, NS, KC * 512], F16, tag="wa")
            wb = wpool.tile([128, NS, KC * 512], F16, tag="wb")
            ba = wpool.tile([128, H], F16, tag="ba")
            bb = wpool.tile([128, H], F16, tag="bb")

            # slot-A weights chunk-by-chunk, then bias, then slot B; xg
            # streaming rides a different queue (gpsimd) so tile 0's
            # tokens aren't stuck behind 8 MB of weights.
            for n in range(NS):
                nc.sync.dma_start(out=wa[:, n, :], in_=wa_d[n, :, :])
            nc.sync.dma_start(out=ba[:, :], in_=ba_d[:, :])
            for n in range(NS):
                nc.sync.dma_start(out=wb[:, n, :], in_=wb_d[n, :, :])
            nc.sync.dma_start(out=bb[:, :], in_=bb_d[:, :])

            for t in range(cfg.t):
                in_a = t < cfg.a
                w_sb, b_sb = (wa, ba) if in_a else (wb, bb)
                src = xga_d[t, :, :] if in_a else xgb_d[t - cfg.a, :, :]
                xt = xpool.tile([128, KC * 128], F16, tag="xt")
                nc.gpsimd.dma_start(out=xt[:, :], in_=src)
                ot = opool.tile([128, H], F16, tag="ot")
                for n in range(NS):
                    ps = pspool.tile([128, 512], F32, tag="ps")
                    for k in range(KC):
                        nc.tensor.matmul(
                            ps[:, :],
                            lhsT=xt[:, k * 128 : (k + 1) * 128],
                            rhs=w_sb[:, n, k * 512 : (k + 1) * 512],
                            start=(k == 0),
                            stop=(k == KC - 1),
                        )
                    nc.vector.tensor_add(
                        out=ot[:, n * 512 : (n + 1) * 512],
                        in0=ps[:, :],
                        in1=b_sb[:, n * 512 : (n + 1) * 512],
                    )
                nc.scalar.dma_start(
                    out=out_d[t * 128 : (t + 1) * 128, :], in_=ot[:, :]
                )

    if fix_waits:
        split_long_waits(nc)
    return nc


# ----------------------------------------------------------------------------
# host-side routing / packing / scatter
# ----------------------------------------------------------------------------


def pack_bins(tile_demand, a, b, n_cores):
    """Assign per-expert 128-token tile demands to 2*n_cores bins
    (n_cores of size `a`, n_cores of size `b`), one expert per bin.

    Returns list of (core, slot, expert, n_tiles) or None if infeasible."""
    bins_a = n_cores
    bins_b = n_cores
    picks = []  # (expert, size_taken, bin_kind)
    for e in sorted(range(len(tile_demand)), key=lambda i: -tile_demand[i]):
        d = tile_demand[e]
        while d > 0:
            if d >= a and bins_a > 0:
                picks.append((e, a, "A"))
                bins_a -= 1
                d -= a
            elif d >= b and bins_b > 0:
                picks.append((e, min(d, b), "B"))
                bins_b -= 1
                d -= b
            elif bins_b > 0:
                picks.append((e, d, "B"))
                bins_b -= 1
                d = 0
            elif bins_a > 0:
                if d > a:
                    return None
                picks.append((e, d, "A"))
                bins_a -= 1
                d = 0
            else:
                return None
    # distribute picks to cores: pair up A picks and B picks per core
    a_picks = [p for p in picks if p[2] == "A"]
    b_picks = [p for p in picks if p[2] == "B"]
    a_picks += [(-1, 0, "A")] * (n_cores - len(a_picks))
    b_picks += [(-1, 0, "B")] * (n_cores - len(b_picks))
    out = []
    for c in range(n_cores):
        ea, na, _ = a_picks[c]
        eb, nb, _ = b_picks[c]
        out.append((c, ea, na, eb, nb))
    return out


def route_and_plan(te, cfg: Cfg):
    """te [N] expert ids -> per-core slot plan with token lists.

    Returns (cfg, plan) where plan[core] = dict with expert_a, tokens_a
    [a*128] (token ids, pads = -1), expert_b, tokens_b [b*128]."""
    counts = np.bincount(te, minlength=E)
    demand = [int(np.ceil(c / 128)) for c in counts]
    assign = pack_bins(demand, cfg.a, cfg.b, N_CORES)
    while assign is None:
        # widen capacity until it fits (different compiled program, still
        # correct); never triggers for data matching the spec stats.
        cfg = Cfg(a=cfg.a + 1, b=cfg.b + 1)
        assign = pack_bins(demand, cfg.a, cfg.b, N_CORES)

    by_expert = {e: np.nonzero(te == e)[0] for e in range(E)}
    offs = {e: 0 for e in range(E)}
    plan = []
    for c, ea, na, eb, nb in assign:
        entry = {}
        for slot, e, ntile, cap in (("a", ea, na, cfg.a), ("b", eb, nb, cfg.b)):
            toks = np.full(cap * 128, -1, dtype=np.int64)
            if e >= 0 and ntile > 0:
                src = by_expert[e]
                o = offs[e]
                take = min(ntile * 128, len(src) - o)
                toks[:take] = src[o : o + take]
                offs[e] = o + take
            entry[f"expert_{slot}"] = e
            entry[f"tokens_{slot}"] = toks
        plan.append(entry)
    for e in range(E):
        assert offs[e] == len(by_expert[e]), (e, offs[e], len(by_expert[e]))
    return cfg, plan


def _tile_w(Wc):
    """[H, D] merged weight -> [ns, 128, kc*512] fp16 (n-major, k-chunked)."""
    WT = Wc.T.astype(np.float16)  # [D, H]
    return np.ascontiguousarray(
        WT.reshape(KC, 128, NS, 512).transpose(2, 1, 0, 3).reshape(NS, 128, KC * 512)
    )


def _tile_x(x16, toks):
    """tokens [m*128] (pads -1 -> token 0) -> [m, 128, kc*128] fp16."""
    tk = np.where(toks < 0, 0, toks)
    xt = x16[tk]  # [m*128, D]
    m = len(tk) // 128
    # want [t, p, k*128 + j] = x[tk[t*128+j], k*128+p]
    return np.ascontiguousarray(
        xt.reshape(m, 128, KC, 128).transpose(0, 3, 2, 1).reshape(m, 128, KC * 128)
    )


def make_in_maps(x, Ws, bs, We, be, te, cfg: Cfg, plan):
    x16 = x.astype(np.float16)
    w_cache, b_cache = {}, {}

    def slot_wb(e):
        if e not in w_cache:
            if e < 0:
                w_cache[e] = np.zeros((NS, 128, KC * 512), dtype=np.float16)
                b_cache[e] = np.zeros((128, H), dtype=np.float16)
            else:
                w_cache[e] = _tile_w(Ws + We[e])
                b_cache[e] = np.ascontiguousarray(
                    np.broadcast_to((bs + be[e]).astype(np.float16), (128, H))
                )
        return w_cache[e], b_cache[e]

    in_maps = []
    for c in range(N_CORES):
        p = plan[c]
        wa, ba = slot_wb(p["expert_a"])
        wb, bb = slot_wb(p["expert_b"])
        in_maps.append(
            {
                "wa16": wa,
                "wb16": wb,
                "ba16": ba,
                "bb16": bb,
                "xga16": _tile_x(x16, p["tokens_a"]),
                "xgb16": _tile_x(x16, p["tokens_b"]),
            }
        )
    return in_maps


def scatter_out(results, cfg: Cfg, plan):
    out = np.empty((N, H), dtype=np.float32)
    seen = 0
    for c in range(N_CORES):
        rows = results[c]["out"]  # [t*128, H] fp16
        toks = np.concatenate([plan[c]["tokens_a"], plan[c]["tokens_b"]])
        valid = toks >= 0
        out[toks[valid]] = rows[valid].astype(np.float32)
        seen += int(valid.sum())
    assert seen == N, seen
    return out


# ----------------------------------------------------------------------------
# entry point
# ----------------------------------------------------------------------------

_PROGRAM_CACHE = {}


def _get_program(cfg: Cfg):
    key = (cfg.a, cfg.b)
    if key not in _PROGRAM_CACHE:
        _PROGRAM_CACHE[key] = build_program(cfg)
    return _PROGRAM_CACHE[key]


def prepare(x, Ws, bs, We, be, Wr, br):
    """Host prep: route, pack, build program + per-core input maps."""
    x = np.asarray(x, dtype=np.float32)
    Ws = np.asarray(Ws, dtype=np.float32)
    bs = np.asarray(bs, dtype=np.float32)
    We = np.asarray(We, dtype=np.float32)
    be = np.asarray(be, dtype=np.float32)
    Wr = np.asarray(Wr, dtype=np.float32)
    br = np.asarray(br, dtype=np.float32)
    assert x.shape == (N, D)

    logits = x @ Wr.T + br
    te = np.argmax(logits, axis=-1)

    cfg, plan = route_and_plan(te, Cfg())
    nc = _get_program(cfg)
    in_maps = make_in_maps(x, Ws, bs, We, be, te, cfg, plan)
    return nc, in_maps, (cfg, plan)


def finish(results, meta):
    cfg, plan = meta
    return scatter_out(results, cfg, plan)


def kernel(x, Ws, bs, We, be, Wr, br):
    from concourse.bass_utils import run_bass_kernel_spmd

    nc, in_maps, meta = prepare(x, Ws, bs, We, be, Wr, br)
    res = run_bass_kernel_spmd(nc, in_maps, list(range(N_CORES)))
    return finish(res.results, meta)


# revision 6
# speedup vs baseline: 2.2504x; 1.0177x over previous
"""MoE (top-1 routing, E=8) Trainium2 Bass kernel — merged-weight version.

Full-input contract: kernel(**inputs) takes the unsharded numpy inputs of
reference.setup_inputs() and returns the full [N, H] float32 output.

Algebraic core: top-1 routing PARTITIONS tokens by expert, so for a token
routed to expert e,

    out = x @ Ws.T + bs + x @ We[e].T + be[e]
        = x @ (Ws + We[e]).T + (bs + be[e])

i.e. the shared and routed FFNs merge into ONE matmul against per-expert
pre-summed weights (summed on host, free).  Device FLOPs halve vs the
two-pass formulation.

Sharding (8 cores): expert-parallel with load balancing.  The global
expert tile demands (ceil(count_e/128), sum ~131) are bin-packed into
8 cores x two weight slots (slot A: 9 tiles, slot B: 8 tiles) so every
core runs the same static program of T=17 token-tiles: tiles 0..8 use
slot A's resident merged weight, tiles 9..16 slot B's.  Which expert a
slot holds is pure input binding, so one compiled program serves any
routing that fits the (9,8) capacity profile; a fallback solver widens
T if needed (different cache key, still correct).

Device schedule per tile: n-major over 8 H-slices of 512, k-inner over
8 D-chunks of 128 accumulated in PSUM fp32 (measured 216 ns/matmul
sustained, LDWEIGHTS hidden), bias add on DVE, fp16 row stores.

Host: router (tiny), expert sort/pack, fp16 casts + pre-tiling, final
row scatter.  No arithmetic combine on host — device rows ARE the
output rows.
"""

import sys

sys.path.insert(0, "/opt/trn_rl_repo")

from dataclasses import dataclass

import numpy as np

import concourse.bass as bass
import concourse.mybir as mybir
from concourse.tile import TileContext

# ----------------------------------------------------------------------------
# configuration
# ----------------------------------------------------------------------------

N, D, H, E = 16384, 1024, 4096, 8
N_CORES = 8
KC = D // 128          # contraction chunks
NS = H // 512          # output H-slices


@dataclass(frozen=True)
class Cfg:
    a: int = 9             # tiles served by weight slot A
    b: int = 8             # tiles served by weight slot B

    @property
    def t(self):
        return self.a + self.b


F16 = mybir.dt.float16
F32 = mybir.dt.float32

MAX_WAITS = 1


def split_long_waits(nc, max_w: int = MAX_WAITS):
    """walrus TPB_CTRL codegen rejects instructions with multiple sync waits
    (CoreV3GenImpl setupSyncWait).  Tile's exit drain can exceed that; move
    excess waits onto same-engine NoOps inserted just before the offender."""
    n_fix = 0
    for f in nc.m.functions:
        for bb in f.blocks:
            insts = bb.instructions
            new_list = []
            changed = False
            for inst in insts:
                si = inst.sync_info
                if si is not None and len(si.on_wait) > max_w:
                    w = list(si.on_wait)
                    k = 0
                    while len(w) > max_w:
                        chunk, w = w[:max_w], w[max_w:]
                        nop = mybir.InstNoOp(
                            name=f"{inst.name}_waitsplit_{k}",
                            engine=inst.engine,
                            sync_info=mybir.SyncInfo(on_wait=chunk, on_update=[]),
                            bass_nofuse=True,
                        )
                        new_list.append(nop)
                        k += 1
                    inst.sync_info = mybir.SyncInfo(
                        on_wait=w, on_update=list(si.on_update)
                    )
                    n_fix += 1
                    changed = True
                new_list.append(inst)
            if changed:
                bb.instructions = new_list
    return n_fix


# ----------------------------------------------------------------------------
# device program
# ----------------------------------------------------------------------------


def build_program(cfg: Cfg, fix_waits: bool = True):
    nc = bass.Bass()

    # weights per slot, n-major so compute can start after one 1 MB chunk:
    # [ns, 128, kc*512] with [n, p, k*512+j] = (Ws+We[e]).T[k*128+p, n*512+j]
    wa_d = nc.declare_dram_parameter("wa16", [NS, 128, KC * 512], F16, isOutput=False)
    wb_d = nc.declare_dram_parameter("wb16", [NS, 128, KC * 512], F16, isOutput=False)
    ba_d = nc.declare_dram_parameter("ba16", [128, H], F16, isOutput=False)
    bb_d = nc.declare_dram_parameter("bb16", [128, H], F16, isOutput=False)
    # token tiles: [t, p, k*128+m] = x[tok[t*128+m], k*128+p]
    xga_d = nc.declare_dram_parameter("xga16", [cfg.a, 128, KC * 128], F16, isOutput=False)
    xgb_d = nc.declare_dram_parameter("xgb16", [cfg.b, 128, KC * 128], F16, isOutput=False)
    out_d = nc.declare_dram_parameter("out", [cfg.t * 128, H], F16, isOutput=True)

    with TileContext(nc) as tc:
        with (
            tc.tile_pool(name="wres", bufs=1) as wpool,
            tc.tile_pool(name="xstream", bufs=5) as xpool,
            tc.tile_pool(name="ostage", bufs=4) as opool,
            tc.tile_pool(name="ps", bufs=4, space="PSUM") as pspool,
        ):
            wa = wpool.tile([128, NS, KC * 512], F16, tag="wa")
            wb = wpool.tile([128, NS, KC * 512], F16, tag="wb")
            ba = wpool.tile([128, H], F16, tag="ba")
            bb = wpool.tile([128, H], F16, tag="bb")

            # Queue plan (16 shared DMA engines, ~400 GB/s aggregate):
            #   gpsimd: xg token tiles (small, critical for tile 0)
            #   sync:   weight chunks, slot A n-major then slot B
            #   scalar: biases first (must not trail the weights: the
            #           first DVE adds need ba or the psum pool clogs)
            #   scalar: output stores
            xts = []
            for t in range(min(3, cfg.a)):
                xt = xpool.tile([128, KC * 128], F16, tag="xt", name=f"xt{t}")
                nc.gpsimd.dma_start(out=xt[:, :], in_=xga_d[t, :, :])
                xts.append(xt)
            nc.scalar.dma_start(out=ba[:, :], in_=ba_d[:, :])
            for n in range(NS):
                nc.sync.dma_start(out=wa[:, n, :], in_=wa_d[n, :, :])
            nc.scalar.dma_start(out=bb[:, :], in_=bb_d[:, :])
            for n in range(NS):
                nc.sync.dma_start(out=wb[:, n, :], in_=wb_d[n, :, :])

            # first 3 tiles n-outer: each weight chunk feeds 3 tiles'
            # matmuls (5.2 us of PE work per ~2.6 us chunk arrival), so
            # the PE never outruns the weight stream.
            g = len(xts)
            ots = [opool.tile([128, H], F16, tag="ot", name=f"ot{t}") for t in range(g)]
            for n in range(NS):
                for t in range(g):
                    ps = pspool.tile([128, 512], F32, tag="ps")
                    for k in range(KC):
                        nc.tensor.matmul(
                            ps[:, :],
                            lhsT=xts[t][:, k * 128 : (k + 1) * 128],
                            rhs=wa[:, n, k * 512 : (k + 1) * 512],
                            start=(k == 0),
                            stop=(k == KC - 1),
                        )
                    nc.vector.tensor_add(
                        out=ots[t][:, n * 512 : (n + 1) * 512],
                        in0=ps[:, :],
                        in1=ba[:, n * 512 : (n + 1) * 512],
                    )
            for t in range(g):
                nc.scalar.dma_start(
                    out=out_d[t * 128 : (t + 1) * 128, :], in_=ots[t][:, :]
                )

            for t in range(g, cfg.t):
                in_a = t < cfg.a
                w_sb, b_sb = (wa, ba) if in_a else (wb, bb)
                src = xga_d[t, :, :] if in_a else xgb_d[t - cfg.a, :, :]
                xt = xpool.tile([128, KC * 128], F16, tag="xt")
                nc.gpsimd.dma_start(out=xt[:, :], in_=src)
                ot = opool.tile([128, H], F16, tag="ot")
                last = t == cfg.t - 1
                for n in range(NS):
                    ps = pspool.tile([128, 512], F32, tag="ps")
                    for k in range(KC):
                        nc.tensor.matmul(
                            ps[:, :],
                            lhsT=xt[:, k * 128 : (k + 1) * 128],
                            rhs=w_sb[:, n, k * 512 : (k + 1) * 512],
                            start=(k == 0),
                            stop=(k == KC - 1),
                        )
                    nc.vector.tensor_add(
                        out=ot[:, n * 512 : (n + 1) * 512],
                        in0=ps[:, :],
                        in1=b_sb[:, n * 512 : (n + 1) * 512],
                    )
                    if last:
                        # slice stores: don't serialize the whole 1 MB
                        # row store behind the final DVE add
                        nc.scalar.dma_start(
                            out=out_d[
                                t * 128 : (t + 1) * 128, n * 512 : (n + 1) * 512
                            ],
                            in_=ot[:, n * 512 : (n + 1) * 512],
                        )
                if not last:
                    nc.scalar.dma_start(
                        out=out_d[t * 128 : (t + 1) * 128, :], in_=ot[:, :]
                    )

    if fix_waits:
        split_long_waits(nc)
    return nc


# ----------------------------------------------------------------------------
# host-side routing / packing / scatter
# ----------------------------------------------------------------------------


def pack_bins(tile_demand, a, b, n_cores):
    """Assign per-expert 128-token tile demands to 2*n_cores bins
    (n_cores of size `a`, n_cores of size `b`), one expert per bin.

    Returns list of (core, slot, expert, n_tiles) or None if infeasible."""
    bins_a = n_cores
    bins_b = n_cores
    picks = []  # (expert, size_taken, bin_kind)
    for e in sorted(range(len(tile_demand)), key=lambda i: -tile_demand[i]):
        d = tile_demand[e]
        while d > 0:
            if d >= a and bins_a > 0:
                picks.append((e, a, "A"))
                bins_a -= 1
                d -= a
            elif d >= b and bins_b > 0:
                picks.append((e, min(d, b), "B"))
                bins_b -= 1
                d -= b
            elif bins_b > 0:
                picks.append((e, d, "B"))
                bins_b -= 1
                d = 0
            elif bins_a > 0:
                if d > a:
                    return None
                picks.append((e, d, "A"))
                bins_a -= 1
                d = 0
            else:
                return None
    # distribute picks to cores: pair up A picks and B picks per core
    a_picks = [p for p in picks if p[2] == "A"]
    b_picks = [p for p in picks if p[2] == "B"]
    a_picks += [(-1, 0, "A")] * (n_cores - len(a_picks))
    b_picks += [(-1, 0, "B")] * (n_cores - len(b_picks))
    out = []
    for c in range(n_cores):
        ea, na, _ = a_picks[c]
        eb, nb, _ = b_picks[c]
        out.append((c, ea, na, eb, nb))
    return out


def route_and_plan(te, cfg: Cfg):
    """te [N] expert ids -> per-core slot plan with token lists.

    Returns (cfg, plan) where plan[core] = dict with expert_a, tokens_a
    [a*128] (token ids, pads = -1), expert_b, tokens_b [b*128]."""
    counts = np.bincount(te, minlength=E)
    demand = [int(np.ceil(c / 128)) for c in counts]
    assign = pack_bins(demand, cfg.a, cfg.b, N_CORES)
    while assign is None:
        # widen capacity until it fits (different compiled program, still
        # correct); never triggers for data matching the spec stats.
        cfg = Cfg(a=cfg.a + 1, b=cfg.b + 1)
        assign = pack_bins(demand, cfg.a, cfg.b, N_CORES)

    by_expert = {e: np.nonzero(te == e)[0] for e in range(E)}
    offs = {e: 0 for e in range(E)}
    plan = []
    for c, ea, na, eb, nb in assign:
        entry = {}
        for slot, e, ntile, cap in (("a", ea, na, cfg.a), ("b", eb, nb, cfg.b)):
            toks = np.full(cap * 128, -1, dtype=np.int64)
            if e >= 0 and ntile > 0:
                src = by_expert[e]
                o = offs[e]
                take = min(ntile * 128, len(src) - o)
                toks[:take] = src[o : o + take]
                offs[e] = o + take
            entry[f"expert_{slot}"] = e
            entry[f"tokens_{slot}"] = toks
        plan.append(entry)
    for e in range(E):
        assert offs[e] == len(by_expert[e]), (e, offs[e], len(by_expert[e]))
    return cfg, plan


def _tile_w(Wc):
    """[H, D] merged weight -> [ns, 128, kc*512] fp16 (n-major, k-chunked)."""
    WT = Wc.T.astype(np.float16)  # [D, H]
    return np.ascontiguousarray(
        WT.reshape(KC, 128, NS, 512).transpose(2, 1, 0, 3).reshape(NS, 128, KC * 512)
    )


def _tile_x(x16, toks):
    """tokens [m*128] (pads -1 -> token 0) -> [m, 128, kc*128] fp16."""
    tk = np.where(toks < 0, 0, toks)
    xt = x16[tk]  # [m*128, D]
    m = len(tk) // 128
    # want [t, p, k*128 + j] = x[tk[t*128+j], k*128+p]
    return np.ascontiguousarray(
        xt.reshape(m, 128, KC, 128).transpose(0, 3, 2, 1).reshape(m, 128, KC * 128)
    )


def make_in_maps(x, Ws, bs, We, be, te, cfg: Cfg, plan):
    x16 = x.astype(np.float16)
    w_cache, b_cache = {}, {}

    def slot_wb(e):
        if e not in w_cache:
            if e < 0:
                w_cache[e] = np.zeros((NS, 128, KC * 512), dtype=np.float16)
                b_cache[e] = np.zeros((128, H), dtype=np.float16)
            else:
                w_cache[e] = _tile_w(Ws + We[e])
                b_cache[e] = np.ascontiguousarray(
                    np.broadcast_to((bs + be[e]).astype(np.float16), (128, H))
                )
        return w_cache[e], b_cache[e]

    in_maps = []
    for c in range(N_CORES):
        p = plan[c]
        wa, ba = slot_wb(p["expert_a"])
        wb, bb = slot_wb(p["expert_b"])
        in_maps.append(
            {
                "wa16": wa,
                "wb16": wb,
                "ba16": ba,
                "bb16": bb,
                "xga16": _tile_x(x16, p["tokens_a"]),
                "xgb16": _tile_x(x16, p["tokens_b"]),
            }
        )
    return in_maps


def scatter_out(results, cfg: Cfg, plan):
    out = np.empty((N, H), dtype=np.float32)
    seen = 0
    for c in range(N_CORES):
        rows = results[c]["out"]  # [t*128, H] fp16
        toks = np.concatenate([plan[c]["tokens_a"], plan[c]["tokens_b"]])
        valid = toks >= 0
        out[toks[valid]] = rows[valid].astype(np.float32)
        seen += int(valid.sum())
    assert seen == N, seen
    return out


# ----------------------------------------------------------------------------
# entry point
# ----------------------------------------------------------------------------

_PROGRAM_CACHE = {}


def _get_program(cfg: Cfg):
    key = (cfg.a, cfg.b)
    if key not in _PROGRAM_CACHE:
        _PROGRAM_CACHE[key] = build_program(cfg)
    return _PROGRAM_CACHE[key]


def prepare(x, Ws, bs, We, be, Wr, br):
    """Host prep: route, pack, build program + per-core input maps."""
    x = np.asarray(x, dtype=np.float32)
    Ws = np.asarray(Ws, dtype=np.float32)
    bs = np.asarray(bs, dtype=np.float32)
    We = np.asarray(We, dtype=np.float32)
    be = np.asarray(be, dtype=np.float32)
    Wr = np.asarray(Wr, dtype=np.float32)
    br = np.asarray(br, dtype=np.float32)
    assert x.shape == (N, D)

    logits = x @ Wr.T + br
    te = np.argmax(logits, axis=-1)

    cfg, plan = route_and_plan(te, Cfg())
    nc = _get_program(cfg)
    in_maps = make_in_maps(x, Ws, bs, We, be, te, cfg, plan)
    return nc, in_maps, (cfg, plan)


def finish(results, meta):
    cfg, plan = meta
    return scatter_out(results, cfg, plan)


def kernel(x, Ws, bs, We, be, Wr, br):
    from concourse.bass_utils import run_bass_kernel_spmd

    nc, in_maps, meta = prepare(x, Ws, bs, We, be, Wr, br)
    res = run_bass_kernel_spmd(nc, in_maps, list(range(N_CORES)))
    return finish(res.results, meta)


# revision 7
# speedup vs baseline: 2.3387x; 1.0392x over previous
"""MoE (top-1 routing, E=8) Trainium2 Bass kernel — H-sharded merged-weight.

out = x @ (Ws + We[e]).T + (bs + be[e])   (top-1 partition => one matmul)

Sharding: each core owns a 512-wide slice of H and computes it for ALL
tokens, which are globally sorted by expert into `nt` 128-token tiles
(nt = sum_e ceil(count_e/128) ~ 131).  Every core runs the identical
static schedule; the only per-core difference is which weight/bias
columns are bound (pure input binding).  Per tile: 8 accumulating
matmuls (K=1024, N=512) against the resident merged-weight slice of
the tile's expert, a DVE bias add, and a 128 KB row store.

Device residency per core: all 8 experts' merged-weight slices
(8 x 1.05 MB) + bias (1 MB) -- the first matmul is gated on just
1.05 MB of DMA, so there is no weight race at startup.
"""

import sys

sys.path.insert(0, "/opt/trn_rl_repo")

import numpy as np

import concourse.bass as bass
import concourse.mybir as mybir
from concourse.tile import TileContext

N, D, H, E = 16384, 1024, 4096, 8
N_CORES = 8
KC = D // 128
HS = H // N_CORES      # 512: per-core H slice

F16 = mybir.dt.float16
F32 = mybir.dt.float32

MAX_WAITS = 1


def split_long_waits(nc, max_w: int = MAX_WAITS):
    """walrus TPB_CTRL codegen rejects instructions with multiple sync
    waits; move excess waits onto same-engine NoOps."""
    n_fix = 0
    for f in nc.m.functions:
        for bb in f.blocks:
            insts = bb.instructions
            new_list = []
            changed = False
            for inst in insts:
                si = inst.sync_info
                if si is not None and len(si.on_wait) > max_w:
                    w = list(si.on_wait)
                    k = 0
                    while len(w) > max_w:
                        chunk, w = w[:max_w], w[max_w:]
                        nop = mybir.InstNoOp(
                            name=f"{inst.name}_waitsplit_{k}",
                            engine=inst.engine,
                            sync_info=mybir.SyncInfo(on_wait=chunk, on_update=[]),
                            bass_nofuse=True,
                        )
                        new_list.append(nop)
                        k += 1
                    inst.sync_info = mybir.SyncInfo(
                        on_wait=w, on_update=list(si.on_update)
                    )
                    n_fix += 1
                    changed = True
                new_list.append(inst)
            if changed:
                bb.instructions = new_list
    return n_fix


# ----------------------------------------------------------------------------
# device program (static schedule = expert id per token tile)
# ----------------------------------------------------------------------------


def build_program(sched: tuple, fix_waits: bool = True):
    nt = len(sched)
    nc = bass.Bass()

    # [e, p, k*512+j] = (Ws+We[e]).T[k*128+p, core*512+j]
    w_d = nc.declare_dram_parameter("w16", [E, 128, KC * HS], F16, isOutput=False)
    b_d = nc.declare_dram_parameter("b16", [128, E * HS], F16, isOutput=False)
    xg_d = nc.declare_dram_parameter("xg16", [nt, 128, KC * 128], F16, isOutput=False)
    out_d = nc.declare_dram_parameter("out", [nt * 128, HS], F16, isOutput=True)

    first_use = []
    seen = set()
    for e in sched:
        if e not in seen:
            seen.add(e)
            first_use.append(e)
    rest = [e for e in range(E) if e not in seen]

    with TileContext(nc) as tc:
        with (
            tc.tile_pool(name="wres", bufs=1) as wpool,
            tc.tile_pool(name="xstream", bufs=6) as xpool,
            tc.tile_pool(name="ostage", bufs=4) as opool,
            tc.tile_pool(name="ps", bufs=4, space="PSUM") as pspool,
        ):
            w = wpool.tile([128, E, KC * HS], F16, tag="w")
            b = wpool.tile([128, E * HS], F16, tag="b")

            # The sync queue's DMA ring comes up first (~9 us) vs
            # gpsimd's (~13 us): put the first token tiles + first
            # weight slice there so matmul 0 can issue ASAP.  Bias
            # rides the (otherwise idle-at-start) scalar queue; the
            # token stream runs on gpsimd.
            n_pre = min(2, nt)
            xts = {}
            for t in range(n_pre):
                xt = xpool.tile([128, KC * 128], F16, tag="xt", name=f"xt{t}")
                nc.sync.dma_start(out=xt[:, :], in_=xg_d[t, :, :])
                xts[t] = xt
            nc.scalar.dma_start(out=b[:, :], in_=b_d[:, :])
            for e in first_use + rest:
                nc.sync.dma_start(out=w[:, e, :], in_=w_d[e, :, :])
            load_at = {}

            for t in range(nt):
                e = sched[t]
                for el, kind, k in load_at.get(t, ()):
                    if kind == "b":
                        nc.gpsimd.dma_start(
                            out=b[:, el * HS : (el + 1) * HS],
                            in_=b_d[:, el * HS : (el + 1) * HS],
                        )
                    elif kind == "w":
                        nc.gpsimd.dma_start(
                            out=w[:, el, k * HS : (k + 1) * HS],
                            in_=w_d[el, :, k * HS : (k + 1) * HS],
                        )
                    else:
                        nc.gpsimd.dma_start(out=w[:, el, :], in_=w_d[el, :, :])
                if t in xts:
                    xt = xts[t]
                else:
                    xt = xpool.tile([128, KC * 128], F16, tag="xt")
                    nc.gpsimd.dma_start(out=xt[:, :], in_=xg_d[t, :, :])
                ot = opool.tile([128, HS], F16, tag="ot")
                ps = pspool.tile([128, HS], F32, tag="ps")
                for k in range(KC):
                    nc.tensor.matmul(
                        ps[:, :],
                        lhsT=xt[:, k * 128 : (k + 1) * 128],
                        rhs=w[:, e, k * HS : (k + 1) * HS],
                        start=(k == 0),
                        stop=(k == KC - 1),
                    )
                nc.vector.tensor_add(
                    out=ot[:, :],
                    in0=ps[:, :],
                    in1=b[:, e * HS : (e + 1) * HS],
                )
                nc.scalar.dma_start(
                    out=out_d[t * 128 : (t + 1) * 128, :], in_=ot[:, :]
                )

    if fix_waits:
        split_long_waits(nc)
    return nc


# ----------------------------------------------------------------------------
# host-side routing / packing / scatter
# ----------------------------------------------------------------------------


def route(te):
    """-> (sched tuple, tokens [nt*128] with -1 pads)."""
    sched = []
    toks = []
    for e in range(E):
        ids = np.nonzero(te == e)[0]
        if len(ids) == 0:
            continue
        nt_e = int(np.ceil(len(ids) / 128))
        pad = np.full(nt_e * 128, -1, dtype=np.int64)
        pad[: len(ids)] = ids
        sched += [e] * nt_e
        toks.append(pad)
    return tuple(sched), np.concatenate(toks)


def _tile_x(x16, toks):
    tk = np.where(toks < 0, 0, toks)
    xt = x16[tk]  # [nt*128, D]
    m = len(tk) // 128
    return np.ascontiguousarray(
        xt.reshape(m, 128, KC, 128).transpose(0, 3, 2, 1).reshape(m, 128, KC * 128)
    )


def make_in_maps(x, Ws, bs, We, be, toks):
    x16 = x.astype(np.float16)
    xg = _tile_x(x16, toks)
    in_maps = []
    for c in range(N_CORES):
        ws = np.empty((E, 128, KC * HS), dtype=np.float16)
        bias = np.empty((128, E * HS), dtype=np.float16)
        for e in range(E):
            WT = (Ws + We[e]).T[:, c * HS : (c + 1) * HS]  # [D, HS] fp32
            ws[e] = (
                WT.reshape(KC, 128, HS).transpose(1, 0, 2).reshape(128, KC * HS)
            ).astype(np.float16)
            bias[:, e * HS : (e + 1) * HS] = (
                (bs + be[e])[c * HS : (c + 1) * HS].astype(np.float16)
            )
        in_maps.append({"w16": ws, "b16": bias, "xg16": xg})
    return in_maps


def scatter_out(results, toks):
    out = np.empty((N, H), dtype=np.float32)
    valid = toks >= 0
    tv = toks[valid]
    for c in range(N_CORES):
        rows = results[c]["out"]  # [nt*128, HS] fp16
        out[tv, c * HS : (c + 1) * HS] = rows[valid].astype(np.float32)
    return out


# ----------------------------------------------------------------------------
# entry point
# ----------------------------------------------------------------------------

_PROGRAM_CACHE = {}


def _get_program(sched):
    if sched not in _PROGRAM_CACHE:
        _PROGRAM_CACHE[sched] = build_program(sched)
    return _PROGRAM_CACHE[sched]


def prepare(x, Ws, bs, We, be, Wr, br):
    x = np.asarray(x, dtype=np.float32)
    Ws = np.asarray(Ws, dtype=np.float32)
    bs = np.asarray(bs, dtype=np.float32)
    We = np.asarray(We, dtype=np.float32)
    be = np.asarray(be, dtype=np.float32)
    Wr = np.asarray(Wr, dtype=np.float32)
    br = np.asarray(br, dtype=np.float32)
    assert x.shape == (N, D)

    logits = x @ Wr.T + br
    te = np.argmax(logits, axis=-1)
    sched, toks = route(te)
    nc = _get_program(sched)
    in_maps = make_in_maps(x, Ws, bs, We, be, toks)
    return nc, in_maps, toks


def finish(results, toks):
    return scatter_out(results, toks)


def kernel(x, Ws, bs, We, be, Wr, br):
    from concourse.bass_utils import run_bass_kernel_spmd

    nc, in_maps, toks = prepare(x, Ws, bs, We, be, Wr, br)
    res = run_bass_kernel_spmd(nc, in_maps, list(range(N_CORES)))
    return finish(res.results, toks)
